# revision 1
# baseline (speedup 1.0000x reference)
"""Trainium2 Bass kernel for single-head attention with QKV projections.

Reference computation (per batch b):
    Q = x@Wq + bq; K = x@Wk + bk; V = x@Wv + bv          [S, D]
    out = softmax(Q @ K.T / sqrt(D)) @ V                  [S, D]
with B=4, S=2048, D=1024, fp32.

Sharding: 8 cores = 4 batches x 2 query-halves. Each core receives x for its
batch with rows permuted so its own query half comes first (attention is
invariant to key order), computes K/V for the full (permuted) sequence and
Q for rows 0..1023, and returns out rows for its query half.

Per-core schedule (all matmuls in float32r -- full PE rate, ~2e-4 rel err):
  Phase P: load Wq/Wk/Wv; for each 512-row chunk of x: PE-transpose to xT
           (once), project K^T and V (both spilled to DRAM scratch), and
           Q^T (chunks 0-1, kept in SBUF).
  Phase D: for each 512-key chunk: stream K^T/V chunks back; scoresT =
           K^T-cols^T @ Q^T (PSUM), exp via ACT (scale folded in), softmax
           denominators via ones-matmul accumulation, PV accumulated into
           SBUF out_acc; normalization by reciprocal sums is fused into the
           last chunk's evacuation.
Softmax skips the max-subtraction: scores here are bounded (|s| < ~20) so
exp is safely inside fp32 range; softmax(s) = exp(s)/sum(exp(s)) exactly.
"""
import sys

sys.path.insert(0, "/opt/trn_rl_repo")

import numpy as np

import concourse.bass as bass
import concourse.mybir as mybir
import concourse.tile as tile
from concourse import bacc
from concourse.bass_utils import run_bass_kernel_spmd
from concourse.masks import make_identity

F32 = mybir.dt.float32
F32R = mybir.dt.float32r

B, S, D = 4, 2048, 1024
SQ = S // 2          # queries per core
SCALE = 1.0 / float(np.sqrt(D))
CH_P = 512           # projection-pass chunk (rows of x)
CH_K = 512           # attention-pass key chunk
N_CH_P = S // CH_P
N_CH_K = S // CH_K
ET = D // 128        # 128-row tiles along d / e dims


def build():
    nc = bacc.Bacc()
    x = nc.dram_tensor("x", [S, D], F32, kind="ExternalInput")
    wq = nc.dram_tensor("wq", [D, D], F32, kind="ExternalInput")
    wk = nc.dram_tensor("wk", [D, D], F32, kind="ExternalInput")
    wv = nc.dram_tensor("wv", [D, D], F32, kind="ExternalInput")
    bq = nc.dram_tensor("bq", [D], F32, kind="ExternalInput")
    bk = nc.dram_tensor("bk", [D], F32, kind="ExternalInput")
    bv = nc.dram_tensor("bv", [D], F32, kind="ExternalInput")
    out = nc.dram_tensor("out", [SQ, D], F32, kind="ExternalOutput")

    with tile.TileContext(nc) as tc:
        with tc.tile_pool(name="const", bufs=1) as const, \
             tc.tile_pool(name="persist", bufs=1) as persist, \
             tc.tile_pool(name="dram", bufs=1, space="DRAM") as dram:
            ident_f = const.tile([128, 128], F32)
            make_identity(nc, ident_f)
            ident = const.tile([128, 128], F32R)
            nc.vector.tensor_copy(ident, ident_f)
            bq_sb = const.tile([128, ET], F32)
            nc.gpsimd.dma_start(out=bq_sb, in_=bq.ap().rearrange("(t p) -> p t", p=128))
            bk_sb = const.tile([128, ET], F32)
            nc.gpsimd.dma_start(out=bk_sb, in_=bk.ap().rearrange("(t p) -> p t", p=128))
            ones_f = const.tile([128, 1], F32)
            nc.vector.memset(ones_f, 1.0)
            ones = const.tile([128, 1], F32R)
            nc.vector.tensor_copy(ones, ones_f)

            qT = persist.tile([128, ET, SQ], F32R)       # Q^T [e, q], resident
            k_spill = dram.tile([ET, 128, S], F32R)      # K^T as (eo, p, k)
            v_spill = dram.tile([S, D], F32R)            # V rows
            sums_scratch = dram.tile([SQ], F32)

            def load_w(pool, w_dram, name):
                # one DMA per 128-row block: matmuls depending on block `do`
                # can start as soon as that 512KB lands.
                w_sb = pool.tile([128, ET, D], F32R, name=name)
                w3 = w_dram[:, :].rearrange("(t p) e -> p t e", p=128).bitcast(F32R)
                for half in range(2):
                    for do in range(ET):
                        nc.gpsimd.dma_start(
                            out=w_sb[:, do, half * 512:(half + 1) * 512],
                            in_=w3[:, do, half * 512:(half + 1) * 512])
                return w_sb

            # ---------- Phase P: transpose once, project Q/K/V ----------
            with tc.tile_pool(name="wP", bufs=1) as wP, \
                 tc.tile_pool(name="chP", bufs=1) as chP, \
                 tc.tile_pool(name="stP", bufs=1) as stP, \
                 tc.tile_pool(name="psP", bufs=1, space="PSUM") as psP:
                wv_sb = load_w(wP, wv, "wv_sb")
                bv_ap = bv.ap()
                bv_bc = wP.tile([128, D], F32)
                nc.gpsimd.dma_start(out=bv_bc,
                                    in_=bass.AP(tensor=bv_ap.tensor, offset=bv_ap.offset,
                                                ap=[[0, 128], bv_ap.ap[0]]))

                def transpose_chunk(c):
                    xT_c = chP.tile([128, ET, CH_P], F32R, tag="xT", bufs=2,
                                    name=f"xT_{c}")
                    for st in range(CH_P // 128):
                        x_nat = stP.tile([128, D], F32R, tag="xnat", bufs=3,
                                         name=f"xnat_{c}_{st}")
                        r0 = c * CH_P + st * 128
                        for half in range(2):
                            nc.sync.dma_start(
                                out=x_nat[:, half * 512:(half + 1) * 512],
                                in_=x[r0:r0 + 128,
                                      half * 512:(half + 1) * 512].bitcast(F32R))
                        for do in range(ET):
                            tp = psP.tile([128, 128], F32R, tag="tp", bufs=3,
                                          name=f"tp_{c}_{st}_{do}")
                            nc.tensor.transpose(
                                tp, x_nat[:, do * 128:(do + 1) * 128], ident)
                            nc.scalar.copy(out=xT_c[:, do, st * 128:(st + 1) * 128],
                                           in_=tp)
                    return xT_c

                def project_v(c, xT_c):
                    for st in range(CH_P // 128):
                        for dch in range(2):
                            pv = psP.tile([128, 512], F32, tag="proj", bufs=5,
                                          name=f"pv_{c}_{st}_{dch}")
                            for do in range(ET):
                                nc.tensor.matmul(
                                    pv,
                                    xT_c[:, do, st * 128:(st + 1) * 128],
                                    wv_sb[:, do, dch * 512:(dch + 1) * 512],
                                    start=(do == 0), stop=(do == ET - 1))
                            v_stage = stP.tile([128, 512], F32R, tag="vstage",
                                               bufs=6, name=f"vst_{c}_{st}_{dch}")
                            nc.vector.tensor_tensor(
                                out=v_stage, in0=pv,
                                in1=bv_bc[:, dch * 512:(dch + 1) * 512],
                                op=mybir.AluOpType.add)
                            r0 = c * CH_P + st * 128
                            nc.sync.dma_start(
                                out=v_spill[r0:r0 + 128, dch * 512:(dch + 1) * 512],
                                in_=v_stage)

                def project_k(c, xT_c):
                    for eo in range(ET):
                        pk = psP.tile([128, 512], F32, tag="proj", bufs=5,
                                      name=f"pk_{c}_{eo}")
                        for do in range(ET):
                            nc.tensor.matmul(
                                pk,
                                wk_sb[:, do, eo * 128:(eo + 1) * 128],
                                xT_c[:, do, :],
                                start=(do == 0), stop=(do == ET - 1))
                        kst = stP.tile([128, 512], F32R, tag="kst", bufs=6,
                                       name=f"kst_{c}_{eo}")
                        nc.vector.tensor_scalar(
                            out=kst, in0=pk, scalar1=bk_sb[:, eo:eo + 1],
                            scalar2=None, op0=mybir.AluOpType.add)
                        nc.sync.dma_start(
                            out=k_spill[eo, :, c * CH_P:(c + 1) * CH_P], in_=kst)

                def project_q(c, xT_c):
                    for eo in range(ET):
                        pq = psP.tile([128, 512], F32, tag="proj", bufs=5,
                                      name=f"pq_{c}_{eo}")
                        for do in range(ET):
                            nc.tensor.matmul(
                                pq,
                                wq_sb[:, do, eo * 128:(eo + 1) * 128],
                                xT_c[:, do, :],
                                start=(do == 0), stop=(do == ET - 1))
                        nc.vector.tensor_scalar(
                            out=qT[:, eo, c * CH_P:(c + 1) * CH_P], in0=pq,
                            scalar1=bq_sb[:, eo:eo + 1], scalar2=None,
                            op0=mybir.AluOpType.add)

                # op-major order over the query-half chunks so early PE
                # work only needs Wv (first weight to arrive), then Wk, Wq.
                NQ = SQ // CH_P
                xTs = [transpose_chunk(c) for c in range(NQ)]
                wk_sb = load_w(wP, wk, "wk_sb")
                for c in range(NQ):
                    project_v(c, xTs[c])
                wq_sb = load_w(wP, wq, "wq_sb")
                for c in range(NQ):
                    project_k(c, xTs[c])
                for c in range(NQ):
                    project_q(c, xTs[c])
                for c in range(NQ, N_CH_P):
                    xT_c = transpose_chunk(c)
                    project_v(c, xT_c)
                    project_k(c, xT_c)

            # ---------------- Phase D: attention ----------------
            with tc.tile_pool(name="accp", bufs=1) as accp, \
                 tc.tile_pool(name="chD", bufs=1) as chD, \
                 tc.tile_pool(name="stD", bufs=1) as stD, \
                 tc.tile_pool(name="psD", bufs=1, space="PSUM") as psD:
                acc = accp.tile([128, SQ // 128, D], F32)    # out accum [q, d]
                sums_ps = [psD.tile([1, 512], F32, tag="sums", bufs=2,
                                    name=f"sums_{qch}")
                           for qch in range(SQ // 512)]

                KTK = CH_K // 128    # k-subtiles per chunk
                rs = None
                for c in range(N_CH_K):
                    last = c == N_CH_K - 1
                    kT_c = chD.tile([128, ET, CH_K], F32R, tag="kTc", bufs=2,
                                    name=f"kTc_{c}")
                    for eo in range(ET):
                        nc.sync.dma_start(
                            out=kT_c[:, eo, :],
                            in_=k_spill[eo, :, c * CH_K:(c + 1) * CH_K])
                    v_c = [chD.tile([128, KTK, 512], F32R, tag=f"vc{dch}",
                                    bufs=2, name=f"vc_{c}_{dch}")
                           for dch in range(2)]
                    for dch in range(2):
                        for st in range(KTK):
                            r0 = c * CH_K + st * 128
                            nc.sync.dma_start(
                                out=v_c[dch][:, st, :],
                                in_=v_spill[r0:r0 + 128,
                                            dch * 512:(dch + 1) * 512])
                    expT_c = chD.tile([128, KTK, SQ], F32R, tag="expT", bufs=2,
                                      name=f"expT_{c}")
                    for kt in range(KTK):
                        for qch in range(SQ // 512):
                            pqk = psD.tile([128, 512], F32, tag="qk", bufs=3,
                                           name=f"pqk_{c}_{kt}_{qch}")
                            for eo in range(ET):
                                nc.tensor.matmul(
                                    pqk,
                                    kT_c[:, eo, kt * 128:(kt + 1) * 128],
                                    qT[:, eo, qch * 512:(qch + 1) * 512],
                                    start=(eo == 0), stop=(eo == ET - 1))
                            nc.scalar.activation(
                                out=expT_c[:, kt, qch * 512:(qch + 1) * 512],
                                in_=pqk, func=mybir.ActivationFunctionType.Exp,
                                scale=SCALE)
                            nc.tensor.matmul(
                                sums_ps[qch], ones,
                                expT_c[:, kt, qch * 512:(qch + 1) * 512],
                                start=(c == 0 and kt == 0),
                                stop=(last and kt == KTK - 1))

                    if last:
                        # reciprocal softmax denominators, ready before PV
                        sums_sb = stD.tile([1, SQ], F32)
                        for qch in range(SQ // 512):
                            nc.vector.tensor_copy(
                                sums_sb[:, qch * 512:(qch + 1) * 512],
                                sums_ps[qch])
                        nc.sync.dma_start(
                            out=sums_scratch.rearrange("(one q) -> one q", one=1),
                            in_=sums_sb)
                        rs = stD.tile([128, SQ // 128], F32)
                        nc.sync.dma_start(
                            out=rs,
                            in_=sums_scratch.rearrange("(t p) -> p t", p=128))
                        nc.vector.reciprocal(rs, rs)

                    for qt in range(SQ // 128):
                        for dch in range(2):
                            ppv = psD.tile([128, 512], F32, tag="pv", bufs=3,
                                           name=f"ppv_{c}_{qt}_{dch}")
                            for kt in range(KTK):
                                nc.tensor.matmul(
                                    ppv,
                                    expT_c[:, kt, qt * 128:(qt + 1) * 128],
                                    v_c[dch][:, kt, :],
                                    start=(kt == 0), stop=(kt == KTK - 1))
                            sl = slice(dch * 512, (dch + 1) * 512)
                            a_sl = acc[:, qt, sl]
                            if c == 0:
                                nc.vector.tensor_copy(a_sl, ppv)
                            else:
                                nc.vector.tensor_add(a_sl, a_sl, ppv)
                            if last:
                                # final per-q scale split across ACT and DVE
                                # so the two tail chains drain in parallel
                                if (qt * 2 + dch) % 2 == 0:
                                    nc.scalar.mul(out=a_sl, in_=a_sl,
                                                  mul=rs[:, qt:qt + 1])
                                else:
                                    nc.vector.tensor_scalar(
                                        out=a_sl, in0=a_sl,
                                        scalar1=rs[:, qt:qt + 1], scalar2=None,
                                        op0=mybir.AluOpType.mult)
                                nc.sync.dma_start(
                                    out=out[qt * 128:(qt + 1) * 128, sl],
                                    in_=a_sl)
    nc.finalize()
    return nc


_NC_CACHE = {}


def _get_nc():
    if "nc" not in _NC_CACHE:
        _NC_CACHE["nc"] = build()
    return _NC_CACHE["nc"]


def kernel(x, Wq, bq, Wk, bk, Wv, bv):
    x = np.ascontiguousarray(np.asarray(x, dtype=np.float32))
    nc = _get_nc()
    in_maps = []
    for core in range(8):
        b, h = core // 2, core % 2
        mine = x[b, h * SQ:(h + 1) * SQ]
        other = x[b, (1 - h) * SQ:(2 - h) * SQ]
        xp = np.concatenate([mine, other], axis=0)
        in_maps.append({
            "x": xp,
            "wq": np.asarray(Wq, dtype=np.float32),
            "wk": np.asarray(Wk, dtype=np.float32),
            "wv": np.asarray(Wv, dtype=np.float32),
            "bq": np.asarray(bq, dtype=np.float32),
            "bk": np.asarray(bk, dtype=np.float32),
            "bv": np.asarray(bv, dtype=np.float32),
        })
    res = run_bass_kernel_spmd(nc, in_maps, core_ids=list(range(8)))
    out = np.empty((B, S, D), dtype=np.float32)
    for core in range(8):
        b, h = core // 2, core % 2
        out[b, h * SQ:(h + 1) * SQ] = res.results[core]["out"]
    return out



# revision 14
# speedup vs baseline: 1.7373x; 1.7373x over previous
"""Trainium2 Bass kernel for single-head attention with QKV projections.

Reference (per batch b):
    Q = x@Wq + bq; K = x@Wk + bk; V = x@Wv + bv          [S, D]
    out = softmax(Q @ K.T / sqrt(D)) @ V                  [S, D]
with B=4, S=2048, D=1024, fp32.

Sharding: 8 cores = 4 batches x 2 query-halves; rows permuted host-side so
each core's query half comes first (attention is key-order invariant).

Algorithm (mixed fp8-e4m3 with residual compensation; all heavy matmuls run
in DoubleRow perf mode = 2 fp8 contraction planes per instruction):

  Scores use the bilinear identity  QK^T = x A x^T + u 1^T + 1 v^T + c  with
  A = Wq Wk^T (host, fp64->fp32), u/c per-query (cancel in softmax, dropped),
  v = x . (Wk bq) per-key (host, exact, folded into the exp bias). This
  removes the K projection entirely.

  Host supplies hi/lo fp8 pairs (t~ = fp8(t), Rt = fp8(t - t~)) for x^T, A,
  and Wv. On-chip:
    G^T  = A~^T x~q^T + RA^T x~q^T + A~^T Rxq^T     (3-term, exact-ish)
    G~, RG = fp8 hi/lo evac of G
    S^T  = x~^T.T G~^T + x~^T.T RG^T                (2-term; key-side x
                                                     residual dropped)
    P'   = fp8(exp(S*scale + v*scale) - 1)          (the -1 shift centers P
                                                     near 0 for 3x better
                                                     fp8 quantization)
    V    = x~^T.T Wv~ + Rx^T.T Wv~ + x~^T.T RWv     (3-term), V~, RV hi/lo
    PV   = P'^T.T V~ + P'^T.T RV                    (2-term)
    sums = ones.T P'  (+S),  colsum = ones.T (V~ + RV)
    out  = (PV + 1 (x) (colsum + S*bv') + (sums-S) (x) bv') / sums / aV
  with bv' = aV*bv folded via two rank-1 (K=1) matmuls into the PV PSUM
  accumulation, so the final evac is a single per-partition scale by
  1/(sums*aV).

Accuracy (numpy sim of this exact dataflow): rel err ~9.7e-3 vs the 2e-2
gate. Cost model: DoubleRow fp8 = 0.5 cyc/output-col at 256-contraction,
4x cheaper than fp32r/bf16 per unit GEMM.
"""
import sys

sys.path.insert(0, "/opt/trn_rl_repo")

import ml_dtypes
import numpy as np

import concourse.bass as bass
import concourse.mybir as mybir
import concourse.tile as tile
from concourse import bacc
from concourse.bass_utils import run_bass_kernel_spmd

F32 = mybir.dt.float32
F32R = mybir.dt.float32r
F8 = mybir.dt.float8e4
DR = mybir.MatmulPerfMode.DoubleRow
E4NP = ml_dtypes.float8_e4m3  # IEEE bias-8 (max 240) — TRN2's fp8e4

B, S, D = 4, 2048, 1024
SQ = S // 2              # queries per core
ET = D // 128            # 128-wide tiles along d/m/e dims (8)
KT = S // 128            # 128-wide key tiles (16)
SCALE = 1.0 / float(np.sqrt(D))
A_ALPHA = 64.0           # fp8 scale for A = Wq Wk^T
V_ALPHA = 32.0            # fp8 scale for Wv / V
SC_C = SCALE / A_ALPHA   # exp() input scale for score PSUM values


def build():
    nc = bacc.Bacc()
    xt8 = nc.dram_tensor("xt8", [ET, 128, S], F8, kind="ExternalInput")
    rxt8 = nc.dram_tensor("rxt8", [ET, 128, S], F8, kind="ExternalInput")
    a8 = nc.dram_tensor("a8", [ET, 128, D], F8, kind="ExternalInput")
    ra8 = nc.dram_tensor("ra8", [ET, 128, D], F8, kind="ExternalInput")
    wv8 = nc.dram_tensor("wv8", [ET, 128, D], F8, kind="ExternalInput")
    rwv8 = nc.dram_tensor("rwv8", [ET, 128, D], F8, kind="ExternalInput")
    vb = nc.dram_tensor("vb", [128, KT], F32, kind="ExternalInput")
    bvu = nc.dram_tensor("bvu", [1, D], F32, kind="ExternalInput")
    rrow = nc.dram_tensor("rrow", [1, D], F32, kind="ExternalInput")
    out = nc.dram_tensor("out", [SQ, D], F32, kind="ExternalOutput")

    with tile.TileContext(nc) as tc:
        with tc.tile_pool(name="const", bufs=1) as const, \
             tc.tile_pool(name="big", bufs=1) as big, \
             tc.tile_pool(name="stage", bufs=1) as stage, \
             tc.tile_pool(name="dram", bufs=1, space="DRAM") as dram:
            # ---- persistent SBUF tensors ----
            xt_sb = big.tile([128, ET, S], F8, name="xt_sb")
            rxt_sb = big.tile([128, ET, S], F8, name="rxt_sb")
            a_sb = big.tile([128, ET, D], F8, name="a_sb")
            ra_sb = big.tile([128, ET, D], F8, name="ra_sb")
            wv_sb = big.tile([128, ET, D], F8, name="wv_sb")
            rwv_sb = big.tile([128, ET, D], F8, name="rwv_sb")
            g8_sb = big.tile([128, ET, SQ], F8, name="g8_sb")
            rg8_sb = big.tile([128, ET, SQ], F8, name="rg8_sb")
            exp_sb = big.tile([128, KT, SQ], F8, name="exp_sb")
            v8_sb = big.tile([128, KT, D], F8, name="v8_sb")
            rv8_sb = big.tile([128, KT, D], F8, name="rv8_sb")

            vb_sb = const.tile([128, KT], F32, name="vb_sb")
            bvu_sb = const.tile([1, D], F32R, name="bvu_sb")
            r_row = const.tile([1, D], F32R, name="r_row")
            ones_f = const.tile([1, 128], F32, name="ones_f")
            nc.vector.memset(ones_f, 1.0)
            ones_row = const.tile([1, 128], F32R, name="ones_row")
            nc.vector.tensor_copy(ones_row, ones_f)
            # DoubleRow weights need plane-stride % 16B == 0: pad to 16 cols
            ones8_f = const.tile([128, 2, 16], F32, name="ones8_f")
            nc.vector.memset(ones8_f, 1.0)
            ones8 = const.tile([128, 2, 16], F8, name="ones8")
            nc.vector.tensor_copy(ones8, ones8_f)
            scratch = dram.tile([SQ], F32, name="scratch")

            # ---- input DMAs, split across two queues by need-time ----
            # gpsimd queue: G-phase inputs (query-half columns + A pair)
            nc.gpsimd.dma_start(out=vb_sb, in_=vb[:, :])
            for t in range(ET):
                nc.gpsimd.dma_start(out=xt_sb[:, t, 0:SQ], in_=xt8[t, :, 0:SQ])
            for t in range(ET):
                nc.gpsimd.dma_start(out=a_sb[:, t, :], in_=a8[t, :, :])
            for t in range(ET):
                nc.gpsimd.dma_start(out=ra_sb[:, t, :], in_=ra8[t, :, :])
            for t in range(ET):
                nc.gpsimd.dma_start(out=rxt_sb[:, t, 0:SQ], in_=rxt8[t, :, 0:SQ])
            # sync queue: V weights + key-other-half columns
            nc.sync.dma_start(out=bvu_sb, in_=bvu[:, :].bitcast(F32R))
            nc.sync.dma_start(out=r_row, in_=rrow[:, :].bitcast(F32R))
            for t in range(ET):
                nc.sync.dma_start(out=wv_sb[:, t, :], in_=wv8[t, :, :])
            for t in range(ET):
                nc.sync.dma_start(out=rwv_sb[:, t, :], in_=rwv8[t, :, :])
            for t in range(ET):
                nc.sync.dma_start(out=xt_sb[:, t, SQ:S], in_=xt8[t, :, SQ:S])
            for t in range(ET):
                nc.sync.dma_start(out=rxt_sb[:, t, SQ:S], in_=rxt8[t, :, SQ:S])

            with tc.tile_pool(name="psA", bufs=1, space="PSUM") as psA:
                # ---------- Phase G: G^T = (A x_q^T) 3-term ----------
                for mt in range(ET):
                    msl = slice(mt * 128, mt * 128 + 128)
                    for qch in range(2):
                        qsl = slice(qch * 512, qch * 512 + 512)
                        ps = psA.tile([128, 512], F32, tag="s", bufs=2,
                                      name=f"gps_{mt}_{qch}")
                        terms = ((a_sb, xt_sb), (ra_sb, xt_sb), (a_sb, rxt_sb))
                        for ti, (L, R) in enumerate(terms):
                            for t in range(4):
                                nc.tensor.matmul(
                                    ps,
                                    L[:, 2 * t:2 * t + 2, msl],
                                    R[:, 2 * t:2 * t + 2, qsl],
                                    start=(ti == 0 and t == 0),
                                    stop=(ti == 2 and t == 3),
                                    perf_mode=DR)
                        nc.scalar.copy(out=g8_sb[:, mt, qsl], in_=ps)
                        nc.vector.tensor_sub(rg8_sb[:, mt, qsl], ps,
                                             g8_sb[:, mt, qsl])

                # ---------- Phase S/V interleaved over key chunks ----------
                sums_ps = [psA.tile([1, 512], F32, tag="sum", bufs=2,
                                    name=f"sums_{q}") for q in range(2)]
                for kt in range(KT):
                    ksl = slice(kt * 128, kt * 128 + 128)
                    # V rows for this key tile (3-term)
                    for dch in range(2):
                        dsl = slice(dch * 512, dch * 512 + 512)
                        psv = psA.tile([128, 512], F32, tag="v", bufs=2,
                                       name=f"vps_{kt}_{dch}")
                        terms = ((xt_sb, wv_sb), (rxt_sb, wv_sb),
                                 (xt_sb, rwv_sb))
                        for ti, (L, R) in enumerate(terms):
                            for t in range(4):
                                nc.tensor.matmul(
                                    psv,
                                    L[:, 2 * t:2 * t + 2, ksl],
                                    R[:, 2 * t:2 * t + 2, dsl],
                                    start=(ti == 0 and t == 0),
                                    stop=(ti == 2 and t == 3),
                                    perf_mode=DR)
                        nc.scalar.copy(out=v8_sb[:, kt, dsl], in_=psv)
                        nc.vector.tensor_sub(rv8_sb[:, kt, dsl], psv,
                                             v8_sb[:, kt, dsl])
                    # scores^T for this key tile (2-term) -> exp -> P'
                    for qch in range(2):
                        qsl = slice(qch * 512, qch * 512 + 512)
                        pss = psA.tile([128, 512], F32, tag="s", bufs=2,
                                       name=f"sps_{kt}_{qch}")
                        for ti, R in enumerate((g8_sb, rg8_sb)):
                            for t in range(4):
                                nc.tensor.matmul(
                                    pss,
                                    xt_sb[:, 2 * t:2 * t + 2, ksl],
                                    R[:, 2 * t:2 * t + 2, qsl],
                                    start=(ti == 0 and t == 0),
                                    stop=(ti == 1 and t == 3),
                                    perf_mode=DR)
                        est = stage.tile([128, 512], F32, tag="est", bufs=4,
                                         name=f"est_{kt}_{qch}")
                        nc.scalar.activation(
                            out=est, in_=pss,
                            func=mybir.ActivationFunctionType.Exp,
                            bias=vb_sb[:, kt:kt + 1], scale=SC_C)
                        nc.gpsimd.tensor_scalar_sub(
                            exp_sb[:, kt, qsl], est, 1.0)
                    # running softmax denominators per completed key PAIR
                    if kt % 2 == 1:
                        u = kt // 2
                        upl = slice(2 * u, 2 * u + 2)
                        for qch in range(2):
                            qsl = slice(qch * 512, qch * 512 + 512)
                            nc.tensor.matmul(
                                sums_ps[qch], ones8[:, :, 0:1],
                                exp_sb[:, upl, qsl],
                                start=(u == 0), stop=(u == KT // 2 - 1),
                                perf_mode=DR)

                # ---------- softmax denominators / rank-1 rows ----------
                # psum holds sigma' = sum_k P'; true denominators = sigma' + S
                sums_row = stage.tile([1, SQ], F32, name="sums_row")
                for qch in range(2):
                    nc.vector.tensor_copy(
                        sums_row[:, qch * 512:qch * 512 + 512], sums_ps[qch])
                sig_row = stage.tile([1, SQ], F32R, name="sig_row")
                nc.vector.tensor_copy(sig_row, sums_row)
                nc.vector.tensor_scalar_add(sums_row, sums_row, float(S))
                nc.sync.dma_start(
                    out=scratch.rearrange("(o q) -> o q", o=1), in_=sums_row)
                rs = stage.tile([128, ET], F32, name="rs")
                nc.sync.dma_start(
                    out=rs, in_=scratch.rearrange("(t p) -> p t", p=128))
                nc.vector.reciprocal(rs, rs)
                nc.vector.tensor_scalar_mul(rs, rs, 1.0 / V_ALPHA)

            # ---------- Phase PV ----------
            with tc.tile_pool(name="psB", bufs=1, space="PSUM") as psB:
                for qt in range(ET):
                    q0 = qt * 128
                    qtl = slice(q0, q0 + 128)
                    for dch in range(2):
                        dsl = slice(dch * 512, dch * 512 + 512)
                        ps = psB.tile([128, 512], F32, tag="pv", bufs=6,
                                      name=f"pv_{qt}_{dch}")
                        for ti, R in enumerate((v8_sb, rv8_sb)):
                            for u in range(KT // 2):
                                nc.tensor.matmul(
                                    ps,
                                    exp_sb[:, 2 * u:2 * u + 2, qtl],
                                    R[:, 2 * u:2 * u + 2, dsl],
                                    start=(ti == 0 and u == 0), stop=False,
                                    perf_mode=DR)
                        nc.tensor.matmul(ps, ones_row, r_row[:, dsl],
                                         start=False, stop=False)
                        nc.tensor.matmul(ps, sig_row[:, qtl], bvu_sb[:, dsl],
                                         start=False, stop=True)
                        ot = stage.tile([128, 512], F32, tag="ost", bufs=4,
                                        name=f"ot_{qt}_{dch}")
                        nc.vector.tensor_scalar(
                            out=ot, in0=ps, scalar1=rs[:, qt:qt + 1],
                            scalar2=None, op0=mybir.AluOpType.mult)
                        nc.sync.dma_start(out=out[qtl, dsl], in_=ot)
    nc.finalize()
    return nc


_NC_CACHE = {}


def _get_nc():
    if "nc" not in _NC_CACHE:
        _NC_CACHE["nc"] = build()
    return _NC_CACHE["nc"]


def _q8pair(a):
    hi = a.astype(E4NP)
    lo = (a - hi.astype(np.float32)).astype(E4NP)
    return hi, lo


def kernel(x, Wq, bq, Wk, bk, Wv, bv):
    x = np.ascontiguousarray(np.asarray(x, dtype=np.float32))
    Wq = np.asarray(Wq, dtype=np.float32)
    bq = np.asarray(bq, dtype=np.float32)
    Wk = np.asarray(Wk, dtype=np.float32)
    Wv = np.asarray(Wv, dtype=np.float32)
    bv = np.asarray(bv, dtype=np.float32)

    A = (Wq.astype(np.float64) @ Wk.T.astype(np.float64)).astype(np.float32)
    A *= A_ALPHA
    a8, ra8 = _q8pair(A)
    a8 = np.ascontiguousarray(a8.reshape(ET, 128, D))
    ra8 = np.ascontiguousarray(ra8.reshape(ET, 128, D))
    wv8, rwv8 = _q8pair(Wv * V_ALPHA)
    wv8 = np.ascontiguousarray(wv8.reshape(ET, 128, D))
    rwv8 = np.ascontiguousarray(rwv8.reshape(ET, 128, D))
    # per-key score offset v = x . (Wk bq), exact on host; pre-scaled for exp
    v_all = (x.reshape(-1, D) @ (Wk @ bq)).reshape(B, S) * SCALE
    bvu = np.ascontiguousarray((bv * V_ALPHA).reshape(1, D))
    wv8f = wv8.reshape(D, D).astype(np.float32)
    rwv8f = rwv8.reshape(D, D).astype(np.float32)

    nc = _get_nc()
    in_maps = []
    for core in range(8):
        b, h = core // 2, core % 2
        xb = x[b]
        xp = np.concatenate(
            [xb[h * SQ:(h + 1) * SQ], xb[(1 - h) * SQ:(2 - h) * SQ]], axis=0)
        xp8, rxp8 = _q8pair(xp)
        xt = np.ascontiguousarray(xp8.T.reshape(ET, 128, S))
        rxt = np.ascontiguousarray(rxp8.T.reshape(ET, 128, S))
        vp = np.concatenate(
            [v_all[b][h * SQ:(h + 1) * SQ], v_all[b][(1 - h) * SQ:(2 - h) * SQ]])
        vbm = np.ascontiguousarray(vp.reshape(KT, 128).T)
        # host column-sum of on-chip V0 via linearity: sum_k V0[k,:] =
        # (sum_k x~)Wv~ + (sum_k Rx)Wv~ + (sum_k x~)RWv ; plus S*aV*bv
        sx = xp8.astype(np.float32).sum(axis=0)
        srx = rxp8.astype(np.float32).sum(axis=0)
        rrow = (sx @ wv8f + srx @ wv8f + sx @ rwv8f
                + (V_ALPHA * S) * bv).reshape(1, D)
        in_maps.append({
            "xt8": xt, "rxt8": rxt, "a8": a8, "ra8": ra8,
            "wv8": wv8, "rwv8": rwv8, "vb": vbm, "bvu": bvu,
            "rrow": np.ascontiguousarray(rrow.astype(np.float32)),
        })
    res = run_bass_kernel_spmd(nc, in_maps, core_ids=list(range(8)))
    outp = np.empty((B, S, D), dtype=np.float32)
    for core in range(8):
        b, h = core // 2, core % 2
        outp[b, h * SQ:(h + 1) * SQ] = res.results[core]["out"]
    return outp


# revision 24
# speedup vs baseline: 2.2310x; 1.2841x over previous
"""Trainium2 Bass kernel for single-head attention with QKV projections.

Reference (per batch b):
    Q = x@Wq + bq; K = x@Wk + bk; V = x@Wv + bv          [S, D]
    out = softmax(Q @ K.T / sqrt(D)) @ V                  [S, D]
with B=4, S=2048, D=1024, fp32.

Sharding: 8 cores = 4 batches x 2 query-halves; rows permuted host-side so
each core's query half comes first (attention is key-order invariant).

Algorithm (mixed fp8-e4m3 with residual compensation; all heavy matmuls run
in DoubleRow perf mode = 2 fp8 contraction planes per instruction):

  Scores use the bilinear identity  QK^T = x A x^T + u 1^T + 1 v^T + c  with
  A = Wq Wk^T (host, fp64->fp32), u/c per-query (cancel in softmax, dropped),
  v = x . (Wk bq) per-key (host, exact, folded into the exp bias). This
  removes the K projection entirely.

  Host supplies hi/lo fp8 pairs (t~ = fp8(t), Rt = fp8(t - t~)) for x^T, A,
  and Wv. On-chip:
    G^T  = A~^T x~q^T + RA^T x~q^T + A~^T Rxq^T     (3-term, exact-ish)
    G~, RG = fp8 hi/lo evac of G
    S^T  = x~^T.T G~^T + x~^T.T RG^T                (2-term; key-side x
                                                     residual dropped)
    P'   = fp8(exp(S*scale + v*scale) - 1)          (the -1 shift centers P
                                                     near 0 for 3x better
                                                     fp8 quantization)
    V    = x~^T.T Wv~ + Rx^T.T Wv~ + x~^T.T RWv     (3-term), V~, RV hi/lo
    PV   = P'^T.T V~ + P'^T.T RV                    (2-term)
    sums = ones.T P'  (+S),  colsum = ones.T (V~ + RV)
    out  = (PV + 1 (x) (colsum + S*bv') + (sums-S) (x) bv') / sums / aV
  with bv' = aV*bv folded via two rank-1 (K=1) matmuls into the PV PSUM
  accumulation, so the final evac is a single per-partition scale by
  1/(sums*aV).

Accuracy (numpy sim of this exact dataflow): rel err ~9.7e-3 vs the 2e-2
gate. Cost model: DoubleRow fp8 = 0.5 cyc/output-col at 256-contraction,
4x cheaper than fp32r/bf16 per unit GEMM.
"""
import sys

sys.path.insert(0, "/opt/trn_rl_repo")

import ml_dtypes
import numpy as np

import concourse.bass as bass
import concourse.mybir as mybir
import concourse.tile as tile
from concourse import bacc
from concourse.bass_utils import run_bass_kernel_spmd

F32 = mybir.dt.float32
F32R = mybir.dt.float32r
F8 = mybir.dt.float8e4
DR = mybir.MatmulPerfMode.DoubleRow
E4NP = ml_dtypes.float8_e4m3  # IEEE bias-8 (max 240) — TRN2's fp8e4

B, S, D = 4, 2048, 1024
SQ = S // 2              # queries per core
ET = D // 128            # 128-wide tiles along d/m/e dims (8)
KT = S // 128            # 128-wide key tiles (16)
SCALE = 1.0 / float(np.sqrt(D))
A_ALPHA = 64.0           # fp8 scale for A = Wq Wk^T
V_ALPHA = 32.0            # fp8 scale for Wv / V
SC_C = SCALE / A_ALPHA   # exp() input scale for score PSUM values


def build():
    nc = bacc.Bacc()
    xt8 = nc.dram_tensor("xt8", [ET, 128, S], F8, kind="ExternalInput")
    rxt8 = nc.dram_tensor("rxt8", [ET, 128, S], F8, kind="ExternalInput")
    a8 = nc.dram_tensor("a8", [ET, 128, D], F8, kind="ExternalInput")
    ra8 = nc.dram_tensor("ra8", [ET, 128, D], F8, kind="ExternalInput")
    wv8 = nc.dram_tensor("wv8", [ET, 128, D], F8, kind="ExternalInput")
    rwv8 = nc.dram_tensor("rwv8", [ET, 128, D], F8, kind="ExternalInput")
    vb = nc.dram_tensor("vb", [128, KT], F32, kind="ExternalInput")
    crow = nc.dram_tensor("crow", [D], F32, kind="ExternalInput")
    bvr = nc.dram_tensor("bvr", [D], F32, kind="ExternalInput")
    out = nc.dram_tensor("out", [SQ, D], F32, kind="ExternalOutput")

    with tile.TileContext(nc) as tc:
        with tc.tile_pool(name="const", bufs=1) as const, \
             tc.tile_pool(name="big", bufs=1) as big, \
             tc.tile_pool(name="stage", bufs=1) as stage, \
             tc.tile_pool(name="dram", bufs=1, space="DRAM") as dram:
            # ---- persistent SBUF tensors ----
            xt_sb = big.tile([128, ET, S], F8, name="xt_sb")
            rxt_sb = big.tile([128, ET, S], F8, name="rxt_sb")
            a_sb = big.tile([128, ET, D], F8, name="a_sb")
            ra_sb = big.tile([128, ET, D], F8, name="ra_sb")
            wv_sb = big.tile([128, ET, D], F8, name="wv_sb")
            rwv_sb = big.tile([128, ET, D], F8, name="rwv_sb")
            g8_sb = big.tile([128, ET, SQ], F8, name="g8_sb")
            rg8_sb = big.tile([128, ET, SQ], F8, name="rg8_sb")
            exp_sb = big.tile([128, KT, SQ], F8, name="exp_sb")
            v8_sb = big.tile([128, KT, D], F8, name="v8_sb")
            rv8_sb = big.tile([128, KT, D], F8, name="rv8_sb")

            vb_sb = const.tile([128, KT], F32, name="vb_sb")
            cs_bc = const.tile([128, D], F32, name="cs_bc")
            bv_bc = const.tile([128, D], F32, name="bv_bc")
            # DoubleRow weights need plane-stride % 16B == 0: pad to 16 cols
            ones8_f = const.tile([128, 2, 16], F32, name="ones8_f")
            nc.vector.memset(ones8_f, 1.0)
            ones8 = const.tile([128, 2, 16], F8, name="ones8")
            nc.vector.tensor_copy(ones8, ones8_f)
            scratch = dram.tile([SQ], F32, name="scratch")

            # ---- input DMAs: one per tensor-half, spread over 2 queues ----
            def load3(eng, sb, dr, lo, hi):
                eng.dma_start(
                    out=sb[:, :, lo:hi],
                    in_=dr[:, :, :].rearrange("t p s -> p t s")[:, :, lo:hi])

            def bcast(eng, sb, dr):
                ap = dr.ap()
                eng.dma_start(out=sb, in_=bass.AP(
                    tensor=ap.tensor, offset=ap.offset,
                    ap=[[0, 128], ap.ap[0]]))

            # G consumes (qch outer, mt inner): land xt q-half0 + a m-half0
            # first so group (mt0, qch0) can start ~2us in
            load3(nc.sync, xt_sb, xt8, 0, 512)        # xt q-cols 0:512
            load3(nc.gpsimd, a_sb, a8, 0, 512)        # A m-cols 0:512
            load3(nc.sync, ra_sb, ra8, 0, 512)
            load3(nc.gpsimd, rxt_sb, rxt8, 0, 512)
            load3(nc.sync, xt_sb, xt8, 512, SQ)       # xt q-cols 512:1024
            load3(nc.gpsimd, a_sb, a8, 512, D)
            load3(nc.sync, ra_sb, ra8, 512, D)
            load3(nc.gpsimd, rxt_sb, rxt8, 512, SQ)
            nc.sync.dma_start(out=vb_sb, in_=vb[:, :])
            load3(nc.sync, wv_sb, wv8, 0, D)          # V weights
            load3(nc.gpsimd, rwv_sb, rwv8, 0, D)
            load3(nc.sync, xt_sb, xt8, SQ, S)         # key cols, other half
            load3(nc.gpsimd, rxt_sb, rxt8, SQ, S)
            bcast(nc.sync, cs_bc, crow)               # PV-evac constants
            bcast(nc.gpsimd, bv_bc, bvr)

            # ---------- Phase G: G^T = (A x_q^T) 3-term ----------
            with tc.tile_pool(name="psA", bufs=1, space="PSUM") as psA:
                for qch in range(2):
                    qsl = slice(qch * 512, qch * 512 + 512)
                    for mt in range(ET):
                        msl = slice(mt * 128, mt * 128 + 128)
                        ps = psA.tile([128, 512], F32, tag="s", bufs=4,
                                      name=f"gps_{mt}_{qch}")
                        terms = ((a_sb, xt_sb), (ra_sb, xt_sb), (a_sb, rxt_sb))
                        for ti, (L, R) in enumerate(terms):
                            for t in range(4):
                                nc.tensor.matmul(
                                    ps,
                                    L[:, 2 * t:2 * t + 2, msl],
                                    R[:, 2 * t:2 * t + 2, qsl],
                                    start=(ti == 0 and t == 0),
                                    stop=(ti == 2 and t == 3),
                                    perf_mode=DR)
                        nc.scalar.copy(out=g8_sb[:, mt, qsl], in_=ps)
                        nc.vector.tensor_sub(rg8_sb[:, mt, qsl], ps,
                                             g8_sb[:, mt, qsl])

                # ------- Phase S/V interleaved over key chunks -------
                sums_ps = psA.tile([128, ET], F32, tag="sum", bufs=1,
                                   name="sums_ps")
                for kt in range(KT):
                    ksl = slice(kt * 128, kt * 128 + 128)
                    # V rows for this key tile (3-term)
                    for dch in range(2):
                        dsl = slice(dch * 512, dch * 512 + 512)
                        psv = psA.tile([128, 512], F32, tag="v", bufs=3,
                                       name=f"vps_{kt}_{dch}")
                        terms = ((xt_sb, wv_sb), (rxt_sb, wv_sb),
                                 (xt_sb, rwv_sb))
                        for ti, (L, R) in enumerate(terms):
                            for t in range(4):
                                nc.tensor.matmul(
                                    psv,
                                    L[:, 2 * t:2 * t + 2, ksl],
                                    R[:, 2 * t:2 * t + 2, dsl],
                                    start=(ti == 0 and t == 0),
                                    stop=(ti == 2 and t == 3),
                                    perf_mode=DR)
                        nc.scalar.copy(out=v8_sb[:, kt, dsl], in_=psv)
                        nc.vector.tensor_sub(rv8_sb[:, kt, dsl], psv,
                                             v8_sb[:, kt, dsl])
                    # scores^T for this key tile (2-term) -> exp -> P'
                    for qch in range(2):
                        qsl = slice(qch * 512, qch * 512 + 512)
                        pss = psA.tile([128, 512], F32, tag="s", bufs=4,
                                       name=f"sps_{kt}_{qch}")
                        for ti, R in enumerate((g8_sb, rg8_sb)):
                            for t in range(4):
                                nc.tensor.matmul(
                                    pss,
                                    xt_sb[:, 2 * t:2 * t + 2, ksl],
                                    R[:, 2 * t:2 * t + 2, qsl],
                                    start=(ti == 0 and t == 0),
                                    stop=(ti == 1 and t == 3),
                                    perf_mode=DR)
                        est = stage.tile([128, 512], F32, tag="est", bufs=4,
                                         name=f"est_{kt}_{qch}")
                        nc.scalar.activation(
                            out=est, in_=pss,
                            func=mybir.ActivationFunctionType.Exp,
                            bias=vb_sb[:, kt:kt + 1], scale=SC_C)
                        nc.gpsimd.tensor_scalar_sub(
                            exp_sb[:, kt, qsl], est, 1.0)
                    # running softmax denominators, [q-part, qt] layout:
                    # 1-col matmuls, issued one kt late so the exp->sub1
                    # chain is already drained when PE needs the data
                    for ks in ([kt - 1] if 0 < kt < KT - 1 else
                               [kt - 1, kt] if kt == KT - 1 else []):
                        for qt in range(ET):
                            nc.tensor.matmul(
                                sums_ps[:, qt:qt + 1],
                                exp_sb[:, ks, qt * 128:qt * 128 + 128],
                                ones8[:, 0, 0:1],
                                start=(ks == 0), stop=(ks == KT - 1))

                # ---------- softmax denominators / rank-1 rows ----------
                # rs = 1 / ((sigma' + S) * aV), directly in [q-part, qt]
                rs = stage.tile([128, ET], F32, name="rs")
                nc.vector.tensor_scalar(
                    out=rs, in0=sums_ps, scalar1=float(S), scalar2=V_ALPHA,
                    op0=mybir.AluOpType.add, op1=mybir.AluOpType.mult)
                nc.vector.reciprocal(rs, rs)

                # ---------- Phase PV ----------
                # out = (PV' + colsum) * rs_q + bv  (rank-1 bias terms
                # cancel: sig*bvu*rs = bv - S*bvu*rs absorbs the S*bvu)
                for qt in range(ET):
                    q0 = qt * 128
                    qtl = slice(q0, q0 + 128)
                    for dch in range(2):
                        dsl = slice(dch * 512, dch * 512 + 512)
                        ps = psA.tile([128, 512], F32, tag="s", bufs=4,
                                      name=f"pv_{qt}_{dch}")
                        for ti, R in enumerate((v8_sb, rv8_sb)):
                            for u in range(KT // 2):
                                nc.tensor.matmul(
                                    ps,
                                    exp_sb[:, 2 * u:2 * u + 2, qtl],
                                    R[:, 2 * u:2 * u + 2, dsl],
                                    start=(ti == 0 and u == 0),
                                    stop=(ti == 1 and u == KT // 2 - 1),
                                    perf_mode=DR)
                        # cw = colsum*rs_q + bv on Pool (SBUF-only), then
                        # out = psum*rs_q + cw on DVE
                        cw = stage.tile([128, 512], F32, tag="cw", bufs=4,
                                        name=f"cw_{qt}_{dch}")
                        nc.vector.scalar_tensor_tensor(
                            out=cw, in0=cs_bc[:, dsl], scalar=rs[:, qt:qt + 1],
                            in1=bv_bc[:, dsl], op0=mybir.AluOpType.mult,
                            op1=mybir.AluOpType.add)
                        ot = stage.tile([128, 512], F32, tag="ost", bufs=4,
                                        name=f"ot_{qt}_{dch}")
                        nc.vector.scalar_tensor_tensor(
                            out=ot, in0=ps, scalar=rs[:, qt:qt + 1], in1=cw,
                            op0=mybir.AluOpType.mult,
                            op1=mybir.AluOpType.add)
                        eng = nc.sync if (qt * 2 + dch) % 2 == 0 else nc.scalar
                        eng.dma_start(out=out[qtl, dsl], in_=ot)
    nc.finalize()
    return nc


_NC_CACHE = {}


def _get_nc():
    if "nc" not in _NC_CACHE:
        _NC_CACHE["nc"] = build()
    return _NC_CACHE["nc"]


def _q8pair(a):
    hi = a.astype(E4NP)
    lo = (a - hi.astype(np.float32)).astype(E4NP)
    return hi, lo


def kernel(x, Wq, bq, Wk, bk, Wv, bv):
    x = np.ascontiguousarray(np.asarray(x, dtype=np.float32))
    Wq = np.asarray(Wq, dtype=np.float32)
    bq = np.asarray(bq, dtype=np.float32)
    Wk = np.asarray(Wk, dtype=np.float32)
    Wv = np.asarray(Wv, dtype=np.float32)
    bv = np.asarray(bv, dtype=np.float32)

    A = (Wq.astype(np.float64) @ Wk.T.astype(np.float64)).astype(np.float32)
    A *= A_ALPHA
    a8, ra8 = _q8pair(A)
    a8 = np.ascontiguousarray(a8.reshape(ET, 128, D))
    ra8 = np.ascontiguousarray(ra8.reshape(ET, 128, D))
    wv8, rwv8 = _q8pair(Wv * V_ALPHA)
    wv8 = np.ascontiguousarray(wv8.reshape(ET, 128, D))
    rwv8 = np.ascontiguousarray(rwv8.reshape(ET, 128, D))
    # per-key score offset v = x . (Wk bq), exact on host; pre-scaled for exp
    v_all = (x.reshape(-1, D) @ (Wk @ bq)).reshape(B, S) * SCALE
    wv8f = wv8.reshape(D, D).astype(np.float32)
    rwv8f = rwv8.reshape(D, D).astype(np.float32)

    nc = _get_nc()
    in_maps = []
    for core in range(8):
        b, h = core // 2, core % 2
        xb = x[b]
        xp = np.concatenate(
            [xb[h * SQ:(h + 1) * SQ], xb[(1 - h) * SQ:(2 - h) * SQ]], axis=0)
        xp8, rxp8 = _q8pair(xp)
        xt = np.ascontiguousarray(xp8.T.reshape(ET, 128, S))
        rxt = np.ascontiguousarray(rxp8.T.reshape(ET, 128, S))
        vp = np.concatenate(
            [v_all[b][h * SQ:(h + 1) * SQ], v_all[b][(1 - h) * SQ:(2 - h) * SQ]])
        vbm = np.ascontiguousarray(vp.reshape(KT, 128).T)
        # host column-sum of on-chip V0 via linearity: sum_k V0[k,:] =
        # (sum_k x~)Wv~ + (sum_k Rx)Wv~ + (sum_k x~)RWv
        sx = xp8.astype(np.float32).sum(axis=0)
        srx = rxp8.astype(np.float32).sum(axis=0)
        csum = (sx @ wv8f + srx @ wv8f + sx @ rwv8f).astype(np.float32)
        in_maps.append({
            "xt8": xt, "rxt8": rxt, "a8": a8, "ra8": ra8,
            "wv8": wv8, "rwv8": rwv8, "vb": vbm,
            "crow": np.ascontiguousarray(csum),
            "bvr": np.ascontiguousarray(bv),
        })
    res = run_bass_kernel_spmd(nc, in_maps, core_ids=list(range(8)))
    outp = np.empty((B, S, D), dtype=np.float32)
    for core in range(8):
        b, h = core // 2, core % 2
        outp[b, h * SQ:(h + 1) * SQ] = res.results[core]["out"]
    return outp


# revision 30
# speedup vs baseline: 2.3035x; 1.0325x over previous
"""Trainium2 Bass kernel for single-head attention with QKV projections.

Reference (per batch b):
    Q = x@Wq + bq; K = x@Wk + bk; V = x@Wv + bv          [S, D]
    out = softmax(Q @ K.T / sqrt(D)) @ V                  [S, D]
with B=4, S=2048, D=1024, fp32.

Sharding: 8 cores = 4 batches x 2 query-halves; rows permuted host-side so
each core's query half comes first (attention is key-order invariant).

Algorithm (mixed fp8-e4m3 with residual compensation; all heavy matmuls run
in DoubleRow perf mode = 2 fp8 contraction planes per instruction):

  Scores use the bilinear identity  QK^T = x A x^T + u 1^T + 1 v^T + c  with
  A = Wq Wk^T (host, fp64->fp32), u/c per-query (cancel in softmax, dropped),
  v = x . (Wk bq) per-key (host, exact, folded into the exp bias). This
  removes the K projection entirely.

  Host supplies hi/lo fp8 pairs (t~ = fp8(t), Rt = fp8(t - t~)) for x^T, A,
  and Wv. On-chip:
    G^T  = A~^T x~q^T + RA^T x~q^T + A~^T Rxq^T     (3-term, exact-ish)
    G~, RG = fp8 hi/lo evac of G
    S^T  = x~^T.T G~^T + x~^T.T RG^T                (2-term; key-side x
                                                     residual dropped)
    P'   = fp8(exp(S*scale + v*scale) - 1)          (the -1 shift centers P
                                                     near 0 for 3x better
                                                     fp8 quantization)
    V    = x~^T.T Wv~ + Rx^T.T Wv~ + x~^T.T RWv     (3-term), V~, RV hi/lo
    PV   = P'^T.T V~ + P'^T.T RV                    (2-term)
    sums = ones.T P'  (+S),  colsum = ones.T (V~ + RV)
    out  = (PV + 1 (x) (colsum + S*bv') + (sums-S) (x) bv') / sums / aV
  with bv' = aV*bv folded via two rank-1 (K=1) matmuls into the PV PSUM
  accumulation, so the final evac is a single per-partition scale by
  1/(sums*aV).

Accuracy (numpy sim of this exact dataflow): rel err ~9.7e-3 vs the 2e-2
gate. Cost model: DoubleRow fp8 = 0.5 cyc/output-col at 256-contraction,
4x cheaper than fp32r/bf16 per unit GEMM.
"""
import sys

sys.path.insert(0, "/opt/trn_rl_repo")

import ml_dtypes
import numpy as np

import concourse.bass as bass
import concourse.mybir as mybir
import concourse.tile as tile
from concourse import bacc
from concourse.bass_utils import run_bass_kernel_spmd

F32 = mybir.dt.float32
F32R = mybir.dt.float32r
F8 = mybir.dt.float8e4
DR = mybir.MatmulPerfMode.DoubleRow
E4NP = ml_dtypes.float8_e4m3  # IEEE bias-8 (max 240) — TRN2's fp8e4

B, S, D = 4, 2048, 1024
SQ = S // 2              # queries per core
ET = D // 128            # 128-wide tiles along d/m/e dims (8)
KT = S // 128            # 128-wide key tiles (16)
SCALE = 1.0 / float(np.sqrt(D))
A_ALPHA = 64.0           # fp8 scale for A = Wq Wk^T
V_ALPHA = 32.0            # fp8 scale for Wv / V
SC_C = SCALE / A_ALPHA   # exp() input scale for score PSUM values


def build():
    # all fp8 inputs come host-packed as [128(part), ET, cols] pieces whose
    # bytes exactly match the SBUF destination -> 128 descriptors per DMA
    nc = bacc.Bacc()
    def din(name, cols):
        return nc.dram_tensor(name, [128, ET, cols], F8, kind="ExternalInput")
    xq = [din("xtq0", 512), din("xtq1", 512), din("xtkh", 1024)]
    rxq = [din("rxq0", 512), din("rxq1", 512), din("rxkh", 1024)]
    am = [din("am0", 512), din("am1", 512)]
    ram = [din("ram0", 512), din("ram1", 512)]
    wv8 = din("wv8", D)
    rwv8 = din("rwv8", D)
    vb = nc.dram_tensor("vb", [128, KT], F32, kind="ExternalInput")
    crow = nc.dram_tensor("crow", [D], F32, kind="ExternalInput")
    bvr = nc.dram_tensor("bvr", [D], F32, kind="ExternalInput")
    out = nc.dram_tensor("out", [SQ, D], F32, kind="ExternalOutput")

    with tile.TileContext(nc) as tc:
        with tc.tile_pool(name="const", bufs=1) as const, \
             tc.tile_pool(name="big", bufs=1) as big, \
             tc.tile_pool(name="stage", bufs=1) as stage, \
             tc.tile_pool(name="dram", bufs=1, space="DRAM") as dram:
            # ---- persistent SBUF tensors (pieces mirror dram layout) ----
            xq_sb = [big.tile([128, ET, 512], F8, name="xq0"),
                     big.tile([128, ET, 512], F8, name="xq1"),
                     big.tile([128, ET, 1024], F8, name="xkh")]
            rxq_sb = [big.tile([128, ET, 512], F8, name="rxq0"),
                      big.tile([128, ET, 512], F8, name="rxq1"),
                      big.tile([128, ET, 1024], F8, name="rxkh")]
            am_sb = [big.tile([128, ET, 512], F8, name="am0"),
                     big.tile([128, ET, 512], F8, name="am1")]
            ram_sb = [big.tile([128, ET, 512], F8, name="ram0"),
                      big.tile([128, ET, 512], F8, name="ram1")]
            wv_sb = big.tile([128, ET, D], F8, name="wv_sb")
            rwv_sb = big.tile([128, ET, D], F8, name="rwv_sb")

            def xt_at(kt):
                # (x piece, rx piece, local col offset) holding key tile kt
                if kt < 4:
                    return xq_sb[0], rxq_sb[0], kt * 128
                if kt < 8:
                    return xq_sb[1], rxq_sb[1], (kt - 4) * 128
                return xq_sb[2], rxq_sb[2], (kt - 8) * 128
            g8_sb = big.tile([128, ET, SQ], F8, name="g8_sb")
            rg8_sb = big.tile([128, ET, SQ], F8, name="rg8_sb")
            exp_sb = big.tile([128, KT, SQ], F8, name="exp_sb")
            v8_sb = big.tile([128, KT, D], F8, name="v8_sb")
            rv8_sb = big.tile([128, KT, D], F8, name="rv8_sb")

            vb_sb = const.tile([128, KT], F32, name="vb_sb")
            cs_bc = const.tile([128, D], F32, name="cs_bc")
            bv_bc = const.tile([128, D], F32, name="bv_bc")
            # DoubleRow weights need plane-stride % 16B == 0: pad to 16 cols
            ones8_f = const.tile([128, 2, 16], F32, name="ones8_f")
            nc.vector.memset(ones8_f, 1.0)
            ones8 = const.tile([128, 2, 16], F8, name="ones8")
            nc.vector.tensor_copy(ones8, ones8_f)
            warm_f = const.tile([128, 512], F32, name="warm_f")
            nc.vector.memset(warm_f, 0.0)
            warm = const.tile([128, 512], F32R, name="warm")
            nc.vector.tensor_copy(warm, warm_f)
            scratch = dram.tile([SQ], F32, name="scratch")

            def ld(eng, sb, dr):
                eng.dma_start(out=sb, in_=dr[:, :, :])

            def bcast(eng, sb, dr):
                ap = dr.ap()
                eng.dma_start(out=sb, in_=bass.AP(
                    tensor=ap.tensor, offset=ap.offset,
                    ap=[[0, 128], ap.ap[0]]))

            # G consumes (qch outer, mt inner) with term order
            # (a*x, a*rx, ra*x): queue pieces in first-need order
            ld(nc.sync, xq_sb[0], xq[0])
            ld(nc.gpsimd, am_sb[0], am[0])
            ld(nc.sync, rxq_sb[0], rxq[0])
            ld(nc.gpsimd, am_sb[1], am[1])
            ld(nc.sync, ram_sb[0], ram[0])
            ld(nc.gpsimd, rxq_sb[1], rxq[1])
            ld(nc.sync, ram_sb[1], ram[1])
            ld(nc.gpsimd, xq_sb[1], xq[1])
            nc.sync.dma_start(out=vb_sb, in_=vb[:, :])
            ld(nc.sync, wv_sb, wv8)                   # V weights
            ld(nc.gpsimd, rwv_sb, rwv8)
            ld(nc.sync, xq_sb[2], xq[2])              # key cols, other half
            ld(nc.gpsimd, rxq_sb[2], rxq[2])
            bcast(nc.sync, cs_bc, crow)               # PV-evac constants
            bcast(nc.gpsimd, bv_bc, bvr)

            # ---------- Phase G: G^T = (A x_q^T) 3-term ----------
            with tc.tile_pool(name="psA", bufs=1, space="PSUM") as psA:
                # PE p-state warmup: ~14 dummy matmuls burn the cold/mid
                # ramp while the first input DMAs are still in flight
                wps = psA.tile([128, 512], F32, tag="s", bufs=4, name="wps")
                for w in range(8):
                    nc.tensor.matmul(wps, warm[:, 0:128], warm,
                                     start=(w == 0), stop=(w == 7))
                for qch in range(2):
                    qsl = slice(qch * 512, qch * 512 + 512)
                    for mt in range(ET):
                        lm = slice((mt % 4) * 128, (mt % 4) * 128 + 128)
                        a_p, ra_p = am_sb[mt // 4], ram_sb[mt // 4]
                        x_p, rx_p = xq_sb[qch], rxq_sb[qch]
                        ps = psA.tile([128, 512], F32, tag="s", bufs=4,
                                      name=f"gps_{mt}_{qch}")
                        terms = ((a_p, x_p), (a_p, rx_p), (ra_p, x_p))
                        for ti, (L, R) in enumerate(terms):
                            for t in range(4):
                                nc.tensor.matmul(
                                    ps,
                                    L[:, 2 * t:2 * t + 2, lm],
                                    R[:, 2 * t:2 * t + 2, :],
                                    start=(ti == 0 and t == 0),
                                    stop=(ti == 2 and t == 3),
                                    perf_mode=DR)
                        nc.scalar.copy(out=g8_sb[:, mt, qsl], in_=ps)
                        nc.vector.tensor_sub(rg8_sb[:, mt, qsl], ps,
                                             g8_sb[:, mt, qsl])

                # ------- Phase S/V interleaved over key chunks -------
                sums_ps = psA.tile([128, ET], F32, tag="sum", bufs=1,
                                   name="sums_ps")
                for kt in range(KT):
                    x_p, rx_p, lo = xt_at(kt)
                    lk = slice(lo, lo + 128)
                    # V rows for this key tile (3-term)
                    for dch in range(2):
                        dsl = slice(dch * 512, dch * 512 + 512)
                        psv = psA.tile([128, 512], F32, tag="v", bufs=3,
                                       name=f"vps_{kt}_{dch}")
                        terms = ((x_p, wv_sb), (rx_p, wv_sb), (x_p, rwv_sb))
                        for ti, (L, R) in enumerate(terms):
                            for t in range(4):
                                nc.tensor.matmul(
                                    psv,
                                    L[:, 2 * t:2 * t + 2, lk],
                                    R[:, 2 * t:2 * t + 2, dsl],
                                    start=(ti == 0 and t == 0),
                                    stop=(ti == 2 and t == 3),
                                    perf_mode=DR)
                        nc.scalar.copy(out=v8_sb[:, kt, dsl], in_=psv)
                        nc.vector.tensor_sub(rv8_sb[:, kt, dsl], psv,
                                             v8_sb[:, kt, dsl])
                    # scores^T for this key tile (2-term) -> exp -> P'
                    for qch in range(2):
                        qsl = slice(qch * 512, qch * 512 + 512)
                        pss = psA.tile([128, 512], F32, tag="s", bufs=4,
                                       name=f"sps_{kt}_{qch}")
                        for ti, R in enumerate((g8_sb, rg8_sb)):
                            for t in range(4):
                                nc.tensor.matmul(
                                    pss,
                                    x_p[:, 2 * t:2 * t + 2, lk],
                                    R[:, 2 * t:2 * t + 2, qsl],
                                    start=(ti == 0 and t == 0),
                                    stop=(ti == 1 and t == 3),
                                    perf_mode=DR)
                        est = stage.tile([128, 512], F32, tag="est", bufs=4,
                                         name=f"est_{kt}_{qch}")
                        nc.scalar.activation(
                            out=est, in_=pss,
                            func=mybir.ActivationFunctionType.Exp,
                            bias=vb_sb[:, kt:kt + 1], scale=SC_C)
                        nc.gpsimd.tensor_scalar_sub(
                            exp_sb[:, kt, qsl], est, 1.0)
                    # running softmax denominators, [q-part, qt] layout:
                    # 1-col matmuls, issued one kt late so the exp->sub1
                    # chain is already drained when PE needs the data
                    for ks in ([kt - 1] if 0 < kt < KT - 1 else
                               [kt - 1, kt] if kt == KT - 1 else []):
                        for qt in range(ET):
                            nc.tensor.matmul(
                                sums_ps[:, qt:qt + 1],
                                exp_sb[:, ks, qt * 128:qt * 128 + 128],
                                ones8[:, 0, 0:1],
                                start=(ks == 0), stop=(ks == KT - 1))

                # ---------- softmax denominators / rank-1 rows ----------
                # rs = 1 / ((sigma' + S) * aV), directly in [q-part, qt]
                rs = stage.tile([128, ET], F32, name="rs")
                nc.vector.tensor_scalar(
                    out=rs, in0=sums_ps, scalar1=float(S), scalar2=V_ALPHA,
                    op0=mybir.AluOpType.add, op1=mybir.AluOpType.mult)
                nc.vector.reciprocal(rs, rs)

                # ---------- Phase PV ----------
                # out = (PV' + colsum) * rs_q + bv  (rank-1 bias terms
                # cancel: sig*bvu*rs = bv - S*bvu*rs absorbs the S*bvu)
                for qt in range(ET):
                    q0 = qt * 128
                    qtl = slice(q0, q0 + 128)
                    for dch in range(2):
                        dsl = slice(dch * 512, dch * 512 + 512)
                        ps = psA.tile([128, 512], F32, tag="s", bufs=4,
                                      name=f"pv_{qt}_{dch}")
                        for ti, R in enumerate((v8_sb, rv8_sb)):
                            for u in range(KT // 2):
                                nc.tensor.matmul(
                                    ps,
                                    exp_sb[:, 2 * u:2 * u + 2, qtl],
                                    R[:, 2 * u:2 * u + 2, dsl],
                                    start=(ti == 0 and u == 0),
                                    stop=(ti == 1 and u == KT // 2 - 1),
                                    perf_mode=DR)
                        # cw = colsum*rs_q + bv on Pool (SBUF-only), then
                        # out = psum*rs_q + cw on DVE
                        cw = stage.tile([128, 512], F32, tag="cw", bufs=4,
                                        name=f"cw_{qt}_{dch}")
                        nc.vector.scalar_tensor_tensor(
                            out=cw, in0=cs_bc[:, dsl], scalar=rs[:, qt:qt + 1],
                            in1=bv_bc[:, dsl], op0=mybir.AluOpType.mult,
                            op1=mybir.AluOpType.add)
                        ot = stage.tile([128, 512], F32, tag="ost", bufs=4,
                                        name=f"ot_{qt}_{dch}")
                        nc.vector.scalar_tensor_tensor(
                            out=ot, in0=ps, scalar=rs[:, qt:qt + 1], in1=cw,
                            op0=mybir.AluOpType.mult,
                            op1=mybir.AluOpType.add)
                        eng = nc.sync if (qt * 2 + dch) % 2 == 0 else nc.scalar
                        eng.dma_start(out=out[qtl, dsl], in_=ot)
    nc.finalize()
    return nc


_NC_CACHE = {}


def _get_nc():
    if "nc" not in _NC_CACHE:
        _NC_CACHE["nc"] = build()
    return _NC_CACHE["nc"]


def _q8pair(a):
    hi = a.astype(E4NP)
    lo = (a - hi.astype(np.float32)).astype(E4NP)
    return hi, lo


def kernel(x, Wq, bq, Wk, bk, Wv, bv):
    x = np.ascontiguousarray(np.asarray(x, dtype=np.float32))
    Wq = np.asarray(Wq, dtype=np.float32)
    bq = np.asarray(bq, dtype=np.float32)
    Wk = np.asarray(Wk, dtype=np.float32)
    Wv = np.asarray(Wv, dtype=np.float32)
    bv = np.asarray(bv, dtype=np.float32)

    A = (Wq.astype(np.float64) @ Wk.T.astype(np.float64)).astype(np.float32)
    A *= A_ALPHA
    a8, ra8 = _q8pair(A)
    C = np.ascontiguousarray

    def packp(w):
        # [D, D] -> [128(part), ET, D], then m-halves
        t = np.transpose(w.reshape(ET, 128, D), (1, 0, 2))
        return C(t[:, :, 0:512]), C(t[:, :, 512:D])

    amp, ramp = packp(a8), packp(ra8)
    wv8q, rwv8q = _q8pair(Wv * V_ALPHA)
    wvp = C(np.transpose(wv8q.reshape(ET, 128, D), (1, 0, 2)))
    rwvp = C(np.transpose(rwv8q.reshape(ET, 128, D), (1, 0, 2)))
    # per-key score offset v = x . (Wk bq), exact on host; pre-scaled for exp
    v_all = (x.reshape(-1, D) @ (Wk @ bq)).reshape(B, S) * SCALE
    wv8f = wv8q.astype(np.float32)
    rwv8f = rwv8q.astype(np.float32)

    nc = _get_nc()
    in_maps = []
    for core in range(8):
        b, h = core // 2, core % 2
        xb = x[b]
        xp = np.concatenate(
            [xb[h * SQ:(h + 1) * SQ], xb[(1 - h) * SQ:(2 - h) * SQ]], axis=0)
        xp8, rxp8 = _q8pair(xp)
        xt = np.transpose(xp8.T.reshape(ET, 128, S), (1, 0, 2))
        rxt = np.transpose(rxp8.T.reshape(ET, 128, S), (1, 0, 2))
        vp = np.concatenate(
            [v_all[b][h * SQ:(h + 1) * SQ], v_all[b][(1 - h) * SQ:(2 - h) * SQ]])
        vbm = np.ascontiguousarray(vp.reshape(KT, 128).T)
        # host column-sum of on-chip V0 via linearity: sum_k V0[k,:] =
        # (sum_k x~)Wv~ + (sum_k Rx)Wv~ + (sum_k x~)RWv
        sx = xp8.astype(np.float32).sum(axis=0)
        srx = rxp8.astype(np.float32).sum(axis=0)
        csum = (sx @ wv8f + srx @ wv8f + sx @ rwv8f).astype(np.float32)
        C = np.ascontiguousarray
        in_maps.append({
            "xtq0": C(xt[:, :, 0:512]), "xtq1": C(xt[:, :, 512:1024]),
            "xtkh": C(xt[:, :, 1024:2048]),
            "rxq0": C(rxt[:, :, 0:512]), "rxq1": C(rxt[:, :, 512:1024]),
            "rxkh": C(rxt[:, :, 1024:2048]),
            "am0": amp[0], "am1": amp[1], "ram0": ramp[0], "ram1": ramp[1],
            "wv8": wvp, "rwv8": rwvp, "vb": vbm,
            "crow": C(csum), "bvr": C(bv),
        })
    res = run_bass_kernel_spmd(nc, in_maps, core_ids=list(range(8)))
    outp = np.empty((B, S, D), dtype=np.float32)
    for core in range(8):
        b, h = core // 2, core % 2
        outp[b, h * SQ:(h + 1) * SQ] = res.results[core]["out"]
    return outp
